# revision 6
# baseline (speedup 1.0000x reference)
"""Gumbel top-k (sequential masking) Trainium2 kernel.

Problem: B=64 rows, N=16384, K=16 sequential top-1+mask steps.
  noisy = logits + gumbel; per step j: soft_j = softmax(noisy_masked/TAU),
  select argmax, mask it (add log(eps) ~ -inf); outputs st (one-hot,
  straight-through) and softs, each [K, B, N] f32.

Strategy (data-parallel over batch, 8 rows/core on 8 cores):
  - softmax is shift-invariant: with e = exp(z), z = (logits+gumbel)/TAU,
    soft_j = e_j / S_j where e_j is e with the top-j values zeroed and
    S_j = S_0 - sum(top-j values). Selection order = descending values.
  - Each row (16384) is laid out as 16 SBUF partitions x 1024, so a core's
    8 rows fill all 128 partitions.
  - Per-row top-16 values found with DVE max8/match_replace hierarchically
    (per-partition top-16 -> gather 256 candidates/row via SBUF-SBUF DMA ->
    global top-16). All masking/one-hot is done by *value* (match_replace,
    is_equal) so no index arithmetic is needed; every cross-partition move
    is a bit-exact DMA so float equality is safe.
"""

import numpy as np
from contextlib import ExitStack

import concourse.bacc as bacc
import concourse.bass as bass
import concourse.mybir as mybir
import concourse.tile as tile
from concourse.bass_utils import run_bass_kernel_spmd

F32 = mybir.dt.float32
B, N, NCORES = 64, 16384, 8
R = B // NCORES          # rows per core = 8
QP = 16                  # partitions per row
FREE = N // QP           # 1024
P = 128                  # SBUF partitions
INV_TAU = 1.5            # 1/(2/3), exact in fp32

_module_cache = {}


def _build(K: int):
    nc = bacc.Bacc("TRN2", target_bir_lowering=False, debug=False,
                   num_devices=NCORES)
    lg_d = nc.dram_tensor("logits", [P, FREE], F32, kind="ExternalInput")
    gm_d = nc.dram_tensor("gumbel", [P, FREE], F32, kind="ExternalInput")
    softs_d = nc.dram_tensor("softs", [K, P, FREE], F32, kind="ExternalOutput")
    st_d = nc.dram_tensor("st", [K, P, FREE], F32, kind="ExternalOutput")

    with tile.TileContext(nc) as tc, ExitStack() as ctx:
        io = ctx.enter_context(tc.tile_pool(name="io", bufs=2))
        ep = ctx.enter_context(tc.tile_pool(name="e", bufs=3))
        sp_ = ctx.enter_context(tc.tile_pool(name="small", bufs=1))
        op_s = ctx.enter_context(tc.tile_pool(name="soft", bufs=4))
        op_h = ctx.enter_context(tc.tile_pool(name="hard", bufs=4))
        dp = ctx.enter_context(tc.tile_pool(name="dscratch", bufs=1,
                                            space="DRAM"))

        lg = io.tile([P, FREE], F32, tag="in")
        gm = io.tile([P, FREE], F32, tag="in")
        nc.sync.dma_start(out=lg[:], in_=lg_d.ap())
        nc.sync.dma_start(out=gm[:], in_=gm_d.ap())

        z = ep.tile([P, FREE], F32, tag="e")
        nc.vector.tensor_add(z[:], lg[:], gm[:])
        e0 = ep.tile([P, FREE], F32, tag="e")
        sp = sp_.tile([P, 1], F32, tag="sp")
        nc.scalar.activation(e0[:], z[:], mybir.ActivationFunctionType.Exp,
                             scale=INV_TAU, accum_out=sp[:])

        # per-partition top-16 (two max8 rounds); a row's global top-16 is
        # guaranteed to be inside its 16 partitions' local top-16s.
        m1 = sp_.tile([P, 8], F32, tag="m1")
        nc.vector.max(m1[:], e0[:])
        et = sp_.tile([P, FREE], F32, tag="et")
        nc.vector.match_replace(et[:], m1[:], e0[:], 0.0)
        m2 = sp_.tile([P, 8], F32, tag="m2")
        nc.vector.max(m2[:], et[:])

        # gather each row's 256 candidates into its own partition, via DRAM
        # (SBUF-SBUF partition-crossing DMAs are not reliable; DRAM-side APs
        # may have arbitrary strides)
        sc_m = dp.tile([2, P, 8], F32, tag="sc_m")
        nc.sync.dma_start(out=sc_m[0], in_=m1[:])
        nc.sync.dma_start(out=sc_m[1], in_=m2[:])
        cand = sp_.tile([R, 256], F32, tag="cand")
        nc.sync.dma_start(
            out=cand[:],
            in_=sc_m[:].rearrange("i (r q) j -> r i q j", q=QP))
        g1 = sp_.tile([R, 8], F32, tag="g1")
        nc.vector.max(g1[:], cand[:])
        cand2 = sp_.tile([R, 256], F32, tag="cand2")
        nc.vector.match_replace(cand2[:], g1[:], cand[:], 0.0)
        g2 = sp_.tile([R, 8], F32, tag="g2")
        nc.vector.max(g2[:], cand2[:])

        # row sums: per-partition sums -> one partition per row -> reduce
        sc_sp = dp.tile([P, 1], F32, tag="sc_sp")
        nc.sync.dma_start(out=sc_sp[:], in_=sp[:])
        spT = sp_.tile([R, QP], F32, tag="spT")
        nc.sync.dma_start(out=spT[:],
                          in_=sc_sp[:].rearrange("(r q) o -> r q o", q=QP))

        # rhs[:, 0:16] = top-16 values desc; rhs[:, 16:32] = 1/S_j
        rhs = sp_.tile([R, 32], F32, tag="rhs")
        nc.vector.tensor_copy(rhs[:, 0:8], g1[:])
        nc.vector.tensor_copy(rhs[:, 8:16], g2[:])
        SS = sp_.tile([R, 17], F32, tag="SS")
        nc.vector.tensor_reduce(SS[:, 0:1], spT[:], axis=mybir.AxisListType.X,
                                op=mybir.AluOpType.add)
        for j in range(16):
            nc.vector.tensor_tensor(SS[:, j + 1:j + 2], SS[:, j:j + 1],
                                    rhs[:, j:j + 1], mybir.AluOpType.subtract)
        nc.vector.reciprocal(rhs[:, 16:32], SS[:, 0:16])

        # broadcast per-row (values, reciprocals) to all 16 partitions of the
        # row, via DRAM with a step-0 (replicating) read AP
        sc_rhs = dp.tile([R, 32], F32, tag="sc_rhs")
        nc.sync.dma_start(out=sc_rhs[:], in_=rhs[:])
        vbr = sp_.tile([P, 32], F32, tag="vbr")
        nc.sync.dma_start(out=vbr[:],
                          in_=sc_rhs[:].unsqueeze(1).broadcast_to([R, QP, 32]))

        # match_replace wants 8 search keys; slots 1-7 are -1 (never matches e>0)
        vbx = sp_.tile([P, 8 * 16], F32, tag="vbx")
        nc.vector.memset(vbx[:], -1.0)
        nc.vector.tensor_copy(
            vbx[:].rearrange("p (j a) -> p j a", a=8)[:, :, 0:1],
            vbr[:, 0:16].unsqueeze(2))

        ej = e0
        for j in range(K):
            soft = op_s.tile([P, FREE], F32, tag="soft")
            nc.scalar.activation(soft[:], ej[:],
                                 mybir.ActivationFunctionType.Copy,
                                 scale=vbr[:, 16 + j:17 + j])
            hard = op_h.tile([P, FREE], F32, tag="hard")
            nc.gpsimd.tensor_scalar(hard[:], ej[:], vbr[:, j:j + 1], None,
                                    mybir.AluOpType.is_equal)
            nc.sync.dma_start(out=softs_d.ap()[j], in_=soft[:])
            nc.sync.dma_start(out=st_d.ap()[j], in_=hard[:])
            if j + 1 < K:
                en = ep.tile([P, FREE], F32, tag="e")
                nc.vector.match_replace(en[:], vbx[:, 8 * j:8 * j + 8], ej[:],
                                        0.0)
                ej = en
    nc.compile()
    return nc


def kernel(logits, gumbel, k, trace=False):
    K = int(k)
    logits = np.ascontiguousarray(logits, dtype=np.float32)
    gumbel = np.ascontiguousarray(gumbel, dtype=np.float32)
    if K == 0:
        empty = np.zeros((0, B, N), dtype=np.float32)
        return empty, empty.copy()
    assert 1 <= K <= 16, f"unsupported k={K}"
    assert logits.shape == (B, N) and gumbel.shape == (B, N)

    if K not in _module_cache:
        _module_cache[K] = _build(K)
    nc = _module_cache[K]

    in_maps = []
    for c in range(NCORES):
        sl = slice(c * R, (c + 1) * R)
        in_maps.append({
            "logits": logits[sl].reshape(P, FREE),
            "gumbel": gumbel[sl].reshape(P, FREE),
        })

    res = run_bass_kernel_spmd(nc, in_maps, core_ids=list(range(NCORES)),
                               trace=trace)

    st = np.empty((K, B, N), dtype=np.float32)
    softs = np.empty((K, B, N), dtype=np.float32)
    for c in range(NCORES):
        sl = slice(c * R, (c + 1) * R)
        softs[:, sl, :] = res.results[c]["softs"].reshape(K, R, N)
        st[:, sl, :] = res.results[c]["st"].reshape(K, R, N)

    if trace:
        kernel.last_exec_time_ns = res.exec_time_ns
        kernel.last_results = res
    return st, softs


# revision 7
# speedup vs baseline: 3.2066x; 3.2066x over previous
"""Gumbel top-k (sequential masking) Trainium2 kernel.

Problem: B=64 rows, N=16384, K=16 sequential top-1+mask steps.
  noisy = logits + gumbel; per step j: soft_j = softmax(noisy_masked/TAU),
  select argmax, mask it (add log(eps) ~ -inf); outputs st (one-hot,
  straight-through) and softs, each [K, B, N] f32.

Strategy (data-parallel over batch, 8 rows/core on 8 cores):
  - softmax is shift-invariant: with e = exp(z), z = (logits+gumbel)/TAU,
    soft_j = e_j / S_j where e_j is e with the top-j values zeroed and
    S_j = S_0 - sum(top-j values). Selection order = descending values.
  - Each row (16384) is laid out as 16 SBUF partitions x 1024, so a core's
    8 rows fill all 128 partitions.
  - Per-row top-16 values found with DVE max8/match_replace hierarchically
    (per-partition top-16 -> gather 256 candidates/row via SBUF-SBUF DMA ->
    global top-16). All masking/one-hot is done by *value* (match_replace,
    is_equal) so no index arithmetic is needed; every cross-partition move
    is a bit-exact DMA so float equality is safe.
"""

import numpy as np
from contextlib import ExitStack

import concourse.bacc as bacc
import concourse.bass as bass
import concourse.mybir as mybir
import concourse.tile as tile
from concourse.bass_utils import run_bass_kernel_spmd

F32 = mybir.dt.float32
B, N, NCORES = 64, 16384, 8
R = B // NCORES          # rows per core = 8
QP = 16                  # partitions per row
FREE = N // QP           # 1024
P = 128                  # SBUF partitions
INV_TAU = 1.5            # 1/(2/3), exact in fp32

_module_cache = {}


def _build(K: int):
    nc = bacc.Bacc("TRN2", target_bir_lowering=False, debug=False,
                   num_devices=NCORES)
    lg_d = nc.dram_tensor("logits", [P, FREE], F32, kind="ExternalInput")
    gm_d = nc.dram_tensor("gumbel", [P, FREE], F32, kind="ExternalInput")
    softs_d = nc.dram_tensor("softs", [K, P, FREE], F32, kind="ExternalOutput")
    st_d = nc.dram_tensor("st", [K, P, FREE], F32, kind="ExternalOutput")

    with tile.TileContext(nc) as tc, ExitStack() as ctx:
        io = ctx.enter_context(tc.tile_pool(name="io", bufs=2))
        ep = ctx.enter_context(tc.tile_pool(name="e", bufs=3))
        sp_ = ctx.enter_context(tc.tile_pool(name="small", bufs=1))
        op_s = ctx.enter_context(tc.tile_pool(name="soft", bufs=4))
        op_h = ctx.enter_context(tc.tile_pool(name="hard", bufs=4))
        dp = ctx.enter_context(tc.tile_pool(name="dscratch", bufs=1,
                                            space="DRAM"))

        lg = io.tile([P, FREE], F32, tag="in")
        gm = io.tile([P, FREE], F32, tag="in")
        nc.sync.dma_start(out=lg[:], in_=lg_d.ap())
        nc.sync.dma_start(out=gm[:], in_=gm_d.ap())

        z = ep.tile([P, FREE], F32, tag="e")
        nc.vector.tensor_add(z[:], lg[:], gm[:])
        e0 = ep.tile([P, FREE], F32, tag="e")
        sp = sp_.tile([P, 1], F32, tag="sp")
        nc.scalar.activation(e0[:], z[:], mybir.ActivationFunctionType.Exp,
                             scale=INV_TAU, accum_out=sp[:])

        # per-partition top-16 (two max8 rounds); a row's global top-16 is
        # guaranteed to be inside its 16 partitions' local top-16s.
        m1 = sp_.tile([P, 8], F32, tag="m1")
        nc.vector.max(m1[:], e0[:])
        et = sp_.tile([P, FREE], F32, tag="et")
        nc.vector.match_replace(et[:], m1[:], e0[:], 0.0)
        m2 = sp_.tile([P, 8], F32, tag="m2")
        nc.vector.max(m2[:], et[:])

        # gather each row's 256 candidates into its own partition, via DRAM
        # (SBUF-SBUF partition-crossing DMAs are not reliable; DRAM-side APs
        # may have arbitrary strides)
        sc_m = dp.tile([2, P, 8], F32, tag="sc_m")
        nc.sync.dma_start(out=sc_m[0], in_=m1[:])
        nc.sync.dma_start(out=sc_m[1], in_=m2[:])
        cand = sp_.tile([R, 256], F32, tag="cand")
        nc.sync.dma_start(
            out=cand[:],
            in_=sc_m[:].rearrange("i (r q) j -> r i q j", q=QP))
        g1 = sp_.tile([R, 8], F32, tag="g1")
        nc.vector.max(g1[:], cand[:])
        cand2 = sp_.tile([R, 256], F32, tag="cand2")
        nc.vector.match_replace(cand2[:], g1[:], cand[:], 0.0)
        g2 = sp_.tile([R, 8], F32, tag="g2")
        nc.vector.max(g2[:], cand2[:])

        # row sums: per-partition sums -> one partition per row -> reduce
        sc_sp = dp.tile([P, 1], F32, tag="sc_sp")
        nc.sync.dma_start(out=sc_sp[:], in_=sp[:])
        spT = sp_.tile([R, QP], F32, tag="spT")
        nc.sync.dma_start(out=spT[:],
                          in_=sc_sp[:].rearrange("(r q) o -> r q o", q=QP))

        # rhs[:, 0:16] = top-16 values desc; rhs[:, 16:32] = 1/S_j
        rhs = sp_.tile([R, 32], F32, tag="rhs")
        nc.vector.tensor_copy(rhs[:, 0:8], g1[:])
        nc.vector.tensor_copy(rhs[:, 8:16], g2[:])
        SS = sp_.tile([R, 17], F32, tag="SS")
        nc.vector.tensor_reduce(SS[:, 0:1], spT[:], axis=mybir.AxisListType.X,
                                op=mybir.AluOpType.add)
        for j in range(16):
            nc.vector.tensor_tensor(SS[:, j + 1:j + 2], SS[:, j:j + 1],
                                    rhs[:, j:j + 1], mybir.AluOpType.subtract)
        nc.vector.reciprocal(rhs[:, 16:32], SS[:, 0:16])

        # broadcast per-row (values, reciprocals) to all 16 partitions of the
        # row, via DRAM with a step-0 (replicating) read AP
        sc_rhs = dp.tile([R, 32], F32, tag="sc_rhs")
        nc.sync.dma_start(out=sc_rhs[:], in_=rhs[:])
        vbr = sp_.tile([P, 32], F32, tag="vbr")
        nc.sync.dma_start(out=vbr[:],
                          in_=sc_rhs[:].unsqueeze(1).broadcast_to([R, QP, 32]))

        # match_replace wants 8 search keys; slots 1-7 are -1 (never matches e>0)
        vbx = sp_.tile([P, 8 * 16], F32, tag="vbx")
        nc.vector.memset(vbx[:], -1.0)
        nc.vector.tensor_copy(
            vbx[:].rearrange("p (j a) -> p j a", a=8)[:, :, 0:1],
            vbr[:, 0:16].unsqueeze(2))

        ej = e0
        for j in range(K):
            soft = op_s.tile([P, FREE], F32, tag="soft")
            nc.scalar.activation(soft[:], ej[:],
                                 mybir.ActivationFunctionType.Copy,
                                 scale=vbr[:, 16 + j:17 + j])
            hard = op_h.tile([P, FREE], F32, tag="hard")
            nc.vector.tensor_scalar(hard[:], ej[:], vbr[:, j:j + 1], None,
                                    mybir.AluOpType.is_equal)
            nc.sync.dma_start(out=softs_d.ap()[j], in_=soft[:])
            nc.sync.dma_start(out=st_d.ap()[j], in_=hard[:])
            if j + 1 < K:
                en = ep.tile([P, FREE], F32, tag="e")
                nc.vector.match_replace(en[:], vbx[:, 8 * j:8 * j + 8], ej[:],
                                        0.0)
                ej = en
    nc.compile()
    return nc


def kernel(logits, gumbel, k, trace=False):
    K = int(k)
    logits = np.ascontiguousarray(logits, dtype=np.float32)
    gumbel = np.ascontiguousarray(gumbel, dtype=np.float32)
    if K == 0:
        empty = np.zeros((0, B, N), dtype=np.float32)
        return empty, empty.copy()
    assert 1 <= K <= 16, f"unsupported k={K}"
    assert logits.shape == (B, N) and gumbel.shape == (B, N)

    if K not in _module_cache:
        _module_cache[K] = _build(K)
    nc = _module_cache[K]

    in_maps = []
    for c in range(NCORES):
        sl = slice(c * R, (c + 1) * R)
        in_maps.append({
            "logits": logits[sl].reshape(P, FREE),
            "gumbel": gumbel[sl].reshape(P, FREE),
        })

    res = run_bass_kernel_spmd(nc, in_maps, core_ids=list(range(NCORES)),
                               trace=trace)

    st = np.empty((K, B, N), dtype=np.float32)
    softs = np.empty((K, B, N), dtype=np.float32)
    for c in range(NCORES):
        sl = slice(c * R, (c + 1) * R)
        softs[:, sl, :] = res.results[c]["softs"].reshape(K, R, N)
        st[:, sl, :] = res.results[c]["st"].reshape(K, R, N)

    if trace:
        kernel.last_exec_time_ns = res.exec_time_ns
        kernel.last_results = res
    return st, softs
